# revision 20
# baseline (speedup 1.0000x reference)
"""Trainium2 Bass kernel for CustomStaticEdgeConv (GNN message passing).

out[n] = mean_{e: row[e]=n} relu( concat(x[n], x[col_e]-x[n]) @ W.T + b )

Math restructure:
    z_e = B @ x[col_e] + A @ x[row_e] + b,   A = (W1-W2), B = W2
so per edge the MLP is ONE [128ch -> 128feat] matmul over the packed
vector g_e = concat(x[col_e], x[row_e]) with stationary W_cat = [[B.T],[A.T]]
and a per-feature (per-partition) bias b fused into the ReLU drain.

The host does all index work: edges are sharded by destination node
(6250 nodes per core), nodes are sorted by degree and grouped into
batches of 128 with a shared per-batch group size g (max degree in the
batch, rounded up to even); each node's edges occupy g slots (padding
slots are zero vectors, contributing relu(b) which the host subtracts).
The host packs xg[128ch, tot_slots] bf16 per core; the device is a pure
streaming pipeline with no gathers:

    dma_start                 -> xg segment in SBUF              [DMA]
    matmul(W_cat stationary)  -> z in PSUM fp32                  [PE]
    relu(z + b) drain         -> M bf16 in SBUF            [ACT + DVE]
    tensor_reduce(add, 3D AP) -> R per node (bf16)        [DVE + Pool]
    dma out                   -> rout[128, nbatch*128] in DRAM   [DMA]

The elementwise work (drain, segmented reduce) is load-balanced across
the Scalar, Vector, and GpSimd engines; per-engine semaphores track
completion counts so consumers wait on exactly the producers they need.

Host post: out[node] = (R[rank] - pad*relu(b)) * (1/deg), reordered.
"""

import sys

sys.path.insert(0, "/opt/trn_rl_repo")

import numpy as np
import ml_dtypes

import concourse.bass as bass  # noqa: F401
import concourse.bacc as bacc
import concourse.mybir as mybir
from concourse.bass_utils import run_bass_kernel_spmd
from concourse.library_config import standard as standard_lib

# ---------------------------------------------------------------- constants
N_NODES = 50000
F_IN = 64
F_OUT = 128
NCORES = 8
LPC = N_NODES // NCORES          # 6250 nodes per core
NBATCH = (LPC + 127) // 128      # 49 batches of 128 nodes
LPAD = NBATCH * 128              # 6272 node ranks (incl. pad ranks)

SEG_CAPS = [4096, 8192]          # first segments small for fast pipeline start
SEG_SLOTS = 12288                # steady-state segment cap (slots)
NXGBUF = 4
CHUNK = 2048                     # PSUM chunk (columns) = 4 banks
NMSB = 3                         # M buffer depth (batches in flight)
ROUT_GRP = 16                    # batches per output DMA slice

F32 = mybir.dt.float32
BF16 = mybir.dt.bfloat16

ACT, DVE, POOL = "a", "v", "p"


def _drain_eng(t):
    return DVE if t % 7 == 6 else ACT       # ~14% of drains on DVE


def _f1_eng(j):
    return POOL if j % 4 == 1 else DVE      # 25% of fold1 work on GpSimd


# ---------------------------------------------------------------- host prep
def _plan_and_pack(edge_index):
    """Shared SPMD batch plan + per-core packing metadata."""
    rows = np.asarray(edge_index[0], dtype=np.int64)
    cols = np.asarray(edge_index[1], dtype=np.int64)
    core = rows // LPC

    degs = np.zeros((NCORES, LPC), dtype=np.int64)
    per_core_edges = []
    for c in range(NCORES):
        sel = core == c
        loc = (rows[sel] - c * LPC).astype(np.int64)
        cc = cols[sel]
        order = np.argsort(loc, kind="stable")
        loc_s, col_s = loc[order], cc[order]
        deg = np.bincount(loc, minlength=LPC)
        degs[c] = deg
        per_core_edges.append((loc_s, col_s, deg))

    # shared batch plan: g per batch = max over cores of the batch's max
    # degree (deg sorted desc per core), rounded up to even, >= 2
    sorted_degs = -np.sort(-degs, axis=1)
    padded = np.zeros((NCORES, LPAD), dtype=np.int64)
    padded[:, :LPC] = sorted_degs
    gs = []
    for j in range(NBATCH):
        g = int(padded[:, j * 128 : (j + 1) * 128].max())
        g = max(2, (g + 1) & ~1)
        gs.append(g)
    block = [128 * g for g in gs]
    offs = np.concatenate([[0], np.cumsum(block)])
    tot_slots = int(offs[-1])

    # segments: greedy runs of whole batches; first ones small
    segments = []  # (slot_start, nslots, first_batch, nbatches)
    s_start, s_n, s_b0, s_nb = 0, 0, 0, 0
    for j in range(NBATCH):
        cap = SEG_CAPS[len(segments)] if len(segments) < len(SEG_CAPS) else SEG_SLOTS
        if s_n and s_n + block[j] > cap:
            segments.append((s_start, s_n, s_b0, s_nb))
            s_start, s_n, s_b0, s_nb = int(offs[j]), 0, j, 0
        s_n += block[j]
        s_nb += 1
    segments.append((s_start, s_n, s_b0, s_nb))

    # chunks: per batch, pieces of <= CHUNK columns
    chunks = []  # (batch, seg, col0_in_seg, moff_in_batch, width)
    seg_of_batch = {}
    for si, (st, ns, b0, nb) in enumerate(segments):
        for j in range(b0, b0 + nb):
            seg_of_batch[j] = si
    cum_chunks = []
    for j in range(NBATCH):
        si = seg_of_batch[j]
        base = int(offs[j]) - segments[si][0]
        done = 0
        while done < block[j]:
            w = min(CHUNK, block[j] - done)
            chunks.append((j, si, base + done, done, w))
            done += w
        cum_chunks.append(len(chunks))
    cum_chunks_of_seg = [cum_chunks[b0 + nb - 1] for (st, ns, b0, nb) in segments]

    # per-engine bookkeeping
    nchunk = len(chunks)
    dr_eng = [_drain_eng(t) for t in range(nchunk)]
    dr_ord = [0] * nchunk            # ordinal within its engine's drains
    cum_dr = {ACT: [0] * NBATCH, DVE: [0] * NBATCH}
    cnt = {ACT: 0, DVE: 0}
    for t, (bj, si, c0, moff, w) in enumerate(chunks):
        e = dr_eng[t]
        dr_ord[t] = cnt[e]
        cnt[e] += 1
        for j in range(bj, NBATCH):
            cum_dr[e][j] = cnt[e]
    f1_eng = [_f1_eng(j) for j in range(NBATCH)]
    f1_ord = [0] * NBATCH
    fcnt = {DVE: 0, POOL: 0}
    for j in range(NBATCH):
        f1_ord[j] = fcnt[f1_eng[j]]
        fcnt[f1_eng[j]] += 1

    plan = dict(gs=gs, offs=offs, tot_slots=tot_slots, segments=segments,
                chunks=chunks, cum_chunks=cum_chunks,
                cum_chunks_of_seg=cum_chunks_of_seg,
                maxblock=max(block), dr_eng=dr_eng, dr_ord=dr_ord,
                cum_dr=cum_dr, f1_eng=f1_eng, f1_ord=f1_ord)

    cores = []
    for c in range(NCORES):
        loc_s, col_s, deg = per_core_edges[c]
        perm = np.argsort(-deg, kind="stable")      # rank -> node
        rank_of = np.empty(LPC, dtype=np.int64)      # node -> rank
        rank_of[perm] = np.arange(LPC)
        g_of_rank = np.repeat(np.asarray(gs, dtype=np.int64), 128)
        off_of_rank = offs[np.arange(LPAD) // 128] + (np.arange(LPAD) % 128) * g_of_rank
        node_off = off_of_rank[rank_of]

        run_start = np.cumsum(deg) - deg
        within = np.arange(len(loc_s)) - run_start[loc_s]
        slot = node_off[loc_s] + within

        scol = np.full(tot_slots, -1, dtype=np.int64)
        srow = np.full(tot_slots, -1, dtype=np.int64)
        scol[slot] = col_s
        srow[slot] = loc_s + c * LPC

        deg_rank = np.zeros(LPAD, dtype=np.int64)
        deg_rank[:LPC] = deg[perm]
        pad_rank = g_of_rank - deg_rank
        invd_rank = 1.0 / np.maximum(deg_rank, 1)

        cores.append(dict(scol=scol, srow=srow, perm=perm,
                          pad_rank=pad_rank, invd_rank=invd_rank))
    return plan, cores


def _build_program(plan):
    tot = plan["tot_slots"]
    segs = plan["segments"]
    chunks = plan["chunks"]
    cum_chunks = plan["cum_chunks"]
    cum_chunks_of_seg = plan["cum_chunks_of_seg"]
    gs = plan["gs"]
    maxblock = plan["maxblock"]
    dr_eng, dr_ord = plan["dr_eng"], plan["dr_ord"]
    cum_dr = plan["cum_dr"]
    f1_eng, f1_ord = plan["f1_eng"], plan["f1_ord"]
    nseg = len(segs)
    nchunk = len(chunks)

    nc = bacc.Bacc("TRN2")
    xg_d = nc.dram_tensor("xg", [128, tot], BF16, kind="ExternalInput")
    wcat_d = nc.dram_tensor("wcat", [128, 128], BF16, kind="ExternalInput")
    bias_d = nc.dram_tensor("bias", [128, 1], F32, kind="ExternalInput")
    rout_d = nc.dram_tensor("rout", [128, LPAD], BF16, kind="ExternalOutput")

    from contextlib import ExitStack

    with ExitStack() as ctx:
        block = ctx.enter_context(nc.Block())
        sb = lambda name, shape, dt: ctx.enter_context(nc.sbuf_tensor(name, shape, dt))
        ps = lambda name, shape: ctx.enter_context(nc.psum_tensor(name, shape, F32))
        sem = lambda name: ctx.enter_context(nc.semaphore(name))

        xgs = [sb(f"xgs{i}", [128, SEG_SLOTS], BF16) for i in range(NXGBUF)]
        msb = [sb(f"msb{i}", [128, maxblock], BF16) for i in range(NMSB)]
        mf1 = [sb(f"mf1_{i}", [128, maxblock // 2], BF16) for i in range(2)]
        mf2 = [sb(f"mf2_{i}", [128, maxblock // 4], BF16) for i in range(2)]
        rout_sb = sb("rout_sb", [128, LPAD], BF16)
        wcat_s = sb("wcat_s", [128, 128], BF16)
        bias_s = sb("bias_s", [128, 1], F32)
        pz = [ps("pz0", [128, CHUNK]), ps("pz1", [128, CHUNK])]

        s_in = sem("s_in")
        s_seg = [sem(f"s_seg{i}") for i in range(NXGBUF)]
        s_mm = sem("s_mm")
        s_dr = {ACT: sem("s_dr_a"), DVE: sem("s_dr_v")}
        s_f1 = {DVE: sem("s_f1_v"), POOL: sem("s_f1_p")}
        s_red = sem("s_red")
        s_done = sem("s_done")

        # first chunk (within engine e's drain sequence) of each batch
        first_dr_of_batch = {ACT: {}, DVE: {}}
        for t, (bj, si, c0, moff, w) in enumerate(chunks):
            e = dr_eng[t]
            if bj not in first_dr_of_batch[e]:
                first_dr_of_batch[e][bj] = t

        @block.sync
        def _(sync):
            sync.dma_start(wcat_s[:, :], wcat_d[:, :]).then_inc(s_in, 16)
            sync.dma_start(bias_s[:, :], bias_d[:, :]).then_inc(s_in, 16)
            # segment prefetch runs unencumbered: only buffer-recycle waits,
            # so the PE never starves at segment boundaries (HAM stays warm)
            for si, (st, ns, b0, nb) in enumerate(segs):
                if si >= NXGBUF:
                    sync.wait_ge(s_mm, cum_chunks_of_seg[si - NXGBUF])
                sync.dma_start(xgs[si % NXGBUF][:, :ns],
                               xg_d[:, st:st + ns]).then_inc(s_seg[si % NXGBUF], 16)
            # reduce outputs accumulate in rout_sb (never recycled); ship
            # them out in a few overlapped slices as reduces retire
            for j0 in range(0, NBATCH, ROUT_GRP):
                j1 = min(j0 + ROUT_GRP, NBATCH)
                sync.wait_ge(s_red, j1)
                sync.dma_start(rout_d[:, 128 * j0:128 * j1],
                               rout_sb[:, 128 * j0:128 * j1]).then_inc(s_done, 16)

        @block.tensor
        def _(pe):
            pe.wait_ge(s_in, 32)
            for t, (bj, si, c0, moff, w) in enumerate(chunks):
                pe.wait_ge(s_seg[si % NXGBUF], 16 * (si // NXGBUF + 1))
                if t >= 2:
                    tp = t - 2  # pz[t%2] free once chunk t-2 was drained
                    pe.wait_ge(s_dr[dr_eng[tp]], dr_ord[tp] + 1)
                for q0 in range(0, w, 512):
                    qw = min(512, w - q0)
                    mm = pe.matmul(pz[t % 2][:, q0:q0 + qw], wcat_s[:, :],
                                   xgs[si % NXGBUF][:, c0 + q0:c0 + q0 + qw],
                                   start=True, stop=True)
                    if q0 + qw == w:
                        mm.then_inc(s_mm)

        def emit_drain(eng, t):
            bj, si, c0, moff, w = chunks[t]
            eng.wait_ge(s_mm, t + 1)
            if first_dr_of_batch[dr_eng[t]].get(bj) == t and bj >= NMSB:
                jp = bj - NMSB  # msb[bj%NMSB] free once fold1 jp retired
                eng.wait_ge(s_f1[f1_eng[jp]], f1_ord[jp] + 1)
            if dr_eng[t] == ACT:
                eng.activation(msb[bj % NMSB][:, moff:moff + w],
                               pz[t % 2][:, :w],
                               mybir.ActivationFunctionType.Relu,
                               bias=bias_s[:, 0:1]).then_inc(s_dr[ACT])
            else:
                eng.tensor_scalar(msb[bj % NMSB][:, moff:moff + w],
                                  pz[t % 2][:, :w],
                                  bias_s[:, 0:1], 0.0,
                                  op0=mybir.AluOpType.add,
                                  op1=mybir.AluOpType.max).then_inc(s_dr[DVE])

        def emit_fold1(eng, j, self_eng):
            # mf1[j%2] free once reduce j-2 retired; drains of batch j done
            g = gs[j]
            h = g // 2
            if j >= 2:
                eng.wait_ge(s_red, j - 1)
            for e in (ACT, DVE):
                if e != self_eng and cum_dr[e][j]:
                    eng.wait_ge(s_dr[e], cum_dr[e][j])
            m = msb[j % NMSB]
            eng.tensor_tensor(
                mf1[j % 2][:, :128 * h].rearrange("p (n h) -> p n h", h=h),
                m[:, :128 * g].rearrange("p (n g) -> p n g", g=g)[:, :, :h],
                m[:, :128 * g].rearrange("p (n g) -> p n g", g=g)[:, :, h:],
                op=mybir.AluOpType.add,
            ).then_inc(s_f1[self_eng])

        def emit_fold2(dve, j):
            g2 = gs[j] // 2
            h = g2 // 2
            if f1_eng[j] == POOL:
                dve.wait_ge(s_f1[POOL], f1_ord[j] + 1)
            src = mf1[j % 2][:, :128 * g2].rearrange("p (n g) -> p n g", g=g2)
            dve.tensor_tensor(
                mf2[j % 2][:, :128 * h].rearrange("p (n h) -> p n h", h=h),
                src[:, :, :h], src[:, :, h:],
                op=mybir.AluOpType.add,
            )

        def emit_reduce(dve, j):
            g = gs[j]
            two_fold = (g // 2) % 2 == 0 and g >= 4
            gr = g // 4 if two_fold else g // 2
            src = (mf2 if two_fold else mf1)[j % 2]
            if not two_fold and f1_eng[j] == POOL:
                dve.wait_ge(s_f1[POOL], f1_ord[j] + 1)
            dve.tensor_reduce(
                rout_sb[:, 128 * j:128 * (j + 1)],
                src[:, :128 * gr].rearrange("p (n g) -> p n g", g=gr),
                axis=mybir.AxisListType.X,
                op=mybir.AluOpType.add,
            ).then_inc(s_red)

        def emit_steps(dve, j):
            if f1_eng[j] == DVE:
                emit_fold1(dve, j, DVE)
            if (gs[j] // 2) % 2 == 0 and gs[j] >= 4:
                emit_fold2(dve, j)
            emit_reduce(dve, j)

        @block.scalar
        def _(act):
            act.wait_ge(s_in, 32)
            for t in range(nchunk):
                if dr_eng[t] == ACT:
                    emit_drain(act, t)

        @block.vector
        def _(dve):
            dve.wait_ge(s_in, 32)
            with nc.allow_low_precision(reason="bf16 rounding of final sums"):
                done = 0
                for t in range(nchunk):
                    bj = chunks[t][0]
                    while done < bj:
                        emit_steps(dve, done)
                        done += 1
                    if dr_eng[t] == DVE:
                        emit_drain(dve, t)
                while done < NBATCH:
                    emit_steps(dve, done)
                    done += 1

        @block.gpsimd
        def _(gp):
            gp.load_library(standard_lib)
            with nc.allow_low_precision(reason="bf16 folds"):
                for j in range(NBATCH):
                    if f1_eng[j] == POOL:
                        emit_fold1(gp, j, POOL)

    nc.compile()
    return nc


_CACHE = {}
TRACE = False
LAST_EXEC_NS = None
LAST_PROFILE_JSON = None
LAST_TRACE_PATH = None


def kernel(x, edge_index, W, b):
    x = np.asarray(x, dtype=np.float32)
    W = np.asarray(W, dtype=np.float32)
    b = np.asarray(b, dtype=np.float32)
    plan, cores = _plan_and_pack(edge_index)

    key = tuple(plan["gs"])
    if key not in _CACHE:
        _CACHE[key] = _build_program(plan)
    nc = _CACHE[key]

    # stationary weights: rows 0-63 = B = W2.T (x_col), 64-127 = A (x_row)
    W1, W2 = W[:, :F_IN], W[:, F_IN:]
    wcat = np.zeros((128, F_OUT), dtype=np.float32)
    wcat[:F_IN] = W2.T
    wcat[F_IN:] = (W1 - W2).T
    wcat = wcat.astype(ml_dtypes.bfloat16)
    bias = b.reshape(128, 1).astype(np.float32)
    relu_b = np.maximum(b, 0.0)

    xbT = np.ascontiguousarray(x.astype(ml_dtypes.bfloat16).T)  # [64, N]

    tot = plan["tot_slots"]
    in_maps = []
    for c in range(NCORES):
        pc = cores[c]
        scol, srow = pc["scol"], pc["srow"]
        xg = np.zeros((128, tot), dtype=ml_dtypes.bfloat16)
        vs = np.flatnonzero(scol >= 0)
        xg[:F_IN, vs] = xbT[:, scol[vs]]
        xg[F_IN:, vs] = xbT[:, srow[vs]]
        in_maps.append({"xg": xg, "wcat": wcat, "bias": bias})

    global LAST_EXEC_NS, LAST_PROFILE_JSON, LAST_TRACE_PATH
    res = run_bass_kernel_spmd(nc, in_maps, core_ids=list(range(NCORES)),
                               trace=TRACE)
    if TRACE:
        LAST_EXEC_NS = res.exec_time_ns
        LAST_PROFILE_JSON = res.profile_json
        if res.instructions_and_trace is not None:
            LAST_TRACE_PATH = res.instructions_and_trace[1]

    # ---- assembly
    out = np.zeros((N_NODES, F_OUT), dtype=np.float32)
    for c in range(NCORES):
        pc = cores[c]
        R = res.results[c]["rout"].astype(np.float32).T   # [LPAD, 128]
        R = R[:LPC] - pc["pad_rank"][:LPC, None] * relu_b[None, :]
        R *= pc["invd_rank"][:LPC, None]
        out[pc["perm"] + c * LPC] = R
    return out


# revision 22
# speedup vs baseline: 1.1015x; 1.1015x over previous
"""Trainium2 Bass kernel for CustomStaticEdgeConv (GNN message passing).

out[n] = mean_{e: row[e]=n} relu( concat(x[n], x[col_e]-x[n]) @ W.T + b )

Math restructure:
    z_e = B @ x[col_e] + A @ x[row_e] + b,   A = (W1-W2), B = W2
so per edge the MLP is ONE [128ch -> 128feat] matmul over the packed
vector g_e = concat(x[col_e], x[row_e]) with stationary W_cat = [[B.T],[A.T]]
and a per-feature (per-partition) bias b fused into the ReLU drain.

The host does all index work: edges are sharded by destination node
(6250 nodes per core), nodes are sorted by degree and grouped into
batches of 128 with a shared per-batch group size g (max degree in the
batch, rounded up to even); each node's edges occupy g slots (padding
slots are zero vectors, contributing relu(b) which the host subtracts).
The host packs xg[128ch, tot_slots] bf16 per core; the device is a pure
streaming pipeline with no gathers:

    dma_start                 -> xg segment in SBUF              [DMA]
    matmul(W_cat stationary)  -> z in PSUM fp32                  [PE]
    relu(z + b) drain         -> M bf16 in SBUF            [ACT + DVE]
    tensor_reduce(add, 3D AP) -> R per node (bf16)        [DVE + Pool]
    dma out                   -> rout[128, nbatch*128] in DRAM   [DMA]

The elementwise work (drain, segmented reduce) is load-balanced across
the Scalar, Vector, and GpSimd engines; per-engine semaphores track
completion counts so consumers wait on exactly the producers they need.

Host post: out[node] = (R[rank] - pad*relu(b)) * (1/deg), reordered.
"""

import sys

sys.path.insert(0, "/opt/trn_rl_repo")

import numpy as np
import ml_dtypes

import concourse.bass as bass  # noqa: F401
import concourse.bacc as bacc
import concourse.mybir as mybir
from concourse.bass_utils import run_bass_kernel_spmd
from concourse.library_config import standard as standard_lib

# ---------------------------------------------------------------- constants
N_NODES = 50000
F_IN = 64
F_OUT = 128
NCORES = 8
LPC = N_NODES // NCORES          # 6250 nodes per core
NBATCH = (LPC + 127) // 128      # 49 batches of 128 nodes
LPAD = NBATCH * 128              # 6272 node ranks (incl. pad ranks)

SEG_CAPS = [4096, 8192]          # first segments small for fast pipeline start
SEG_SLOTS = 12288                # steady-state segment cap (slots)
NXGBUF = 4
CHUNK = 2048                     # PSUM chunk (columns) = 4 banks
NMSB = 4                         # M buffer depth (batches in flight)
NMF1 = 3                         # fold1 buffer depth
ROUT_GRP = 16                    # batches per output DMA slice

F32 = mybir.dt.float32
BF16 = mybir.dt.bfloat16

ACT, DVE, POOL = "a", "v", "p"


def _drain_eng(t):
    return ACT                              # all drains on Scalar engine


def _f1_eng(j):
    return POOL if j % 2 == 1 else DVE      # 50% of fold1 work on GpSimd


# ---------------------------------------------------------------- host prep
def _plan_and_pack(edge_index):
    """Shared SPMD batch plan + per-core packing metadata."""
    rows = np.asarray(edge_index[0], dtype=np.int64)
    cols = np.asarray(edge_index[1], dtype=np.int64)
    core = rows // LPC

    degs = np.zeros((NCORES, LPC), dtype=np.int64)
    per_core_edges = []
    for c in range(NCORES):
        sel = core == c
        loc = (rows[sel] - c * LPC).astype(np.int64)
        cc = cols[sel]
        order = np.argsort(loc, kind="stable")
        loc_s, col_s = loc[order], cc[order]
        deg = np.bincount(loc, minlength=LPC)
        degs[c] = deg
        per_core_edges.append((loc_s, col_s, deg))

    # shared batch plan: g per batch = max over cores of the batch's max
    # degree (deg sorted desc per core), rounded up to even, >= 2
    sorted_degs = -np.sort(-degs, axis=1)
    padded = np.zeros((NCORES, LPAD), dtype=np.int64)
    padded[:, :LPC] = sorted_degs
    gs = []
    for j in range(NBATCH):
        g = int(padded[:, j * 128 : (j + 1) * 128].max())
        g = max(2, (g + 1) & ~1)
        gs.append(g)
    block = [128 * g for g in gs]
    offs = np.concatenate([[0], np.cumsum(block)])
    tot_slots = int(offs[-1])

    # segments: greedy runs of whole batches; first ones small
    segments = []  # (slot_start, nslots, first_batch, nbatches)
    s_start, s_n, s_b0, s_nb = 0, 0, 0, 0
    for j in range(NBATCH):
        cap = SEG_CAPS[len(segments)] if len(segments) < len(SEG_CAPS) else SEG_SLOTS
        if s_n and s_n + block[j] > cap:
            segments.append((s_start, s_n, s_b0, s_nb))
            s_start, s_n, s_b0, s_nb = int(offs[j]), 0, j, 0
        s_n += block[j]
        s_nb += 1
    segments.append((s_start, s_n, s_b0, s_nb))

    # chunks: per batch, pieces of <= CHUNK columns
    chunks = []  # (batch, seg, col0_in_seg, moff_in_batch, width)
    seg_of_batch = {}
    for si, (st, ns, b0, nb) in enumerate(segments):
        for j in range(b0, b0 + nb):
            seg_of_batch[j] = si
    cum_chunks = []
    for j in range(NBATCH):
        si = seg_of_batch[j]
        base = int(offs[j]) - segments[si][0]
        done = 0
        while done < block[j]:
            w = min(CHUNK, block[j] - done)
            chunks.append((j, si, base + done, done, w))
            done += w
        cum_chunks.append(len(chunks))
    cum_chunks_of_seg = [cum_chunks[b0 + nb - 1] for (st, ns, b0, nb) in segments]

    # per-engine bookkeeping
    nchunk = len(chunks)
    dr_eng = [_drain_eng(t) for t in range(nchunk)]
    dr_ord = [0] * nchunk            # ordinal within its engine's drains
    cum_dr = {ACT: [0] * NBATCH, DVE: [0] * NBATCH}
    cnt = {ACT: 0, DVE: 0}
    for t, (bj, si, c0, moff, w) in enumerate(chunks):
        e = dr_eng[t]
        dr_ord[t] = cnt[e]
        cnt[e] += 1
        for j in range(bj, NBATCH):
            cum_dr[e][j] = cnt[e]
    f1_eng = [_f1_eng(j) for j in range(NBATCH)]
    f1_ord = [0] * NBATCH
    fcnt = {DVE: 0, POOL: 0}
    for j in range(NBATCH):
        f1_ord[j] = fcnt[f1_eng[j]]
        fcnt[f1_eng[j]] += 1

    plan = dict(gs=gs, offs=offs, tot_slots=tot_slots, segments=segments,
                chunks=chunks, cum_chunks=cum_chunks,
                cum_chunks_of_seg=cum_chunks_of_seg,
                maxblock=max(block), dr_eng=dr_eng, dr_ord=dr_ord,
                cum_dr=cum_dr, f1_eng=f1_eng, f1_ord=f1_ord)

    cores = []
    for c in range(NCORES):
        loc_s, col_s, deg = per_core_edges[c]
        perm = np.argsort(-deg, kind="stable")      # rank -> node
        rank_of = np.empty(LPC, dtype=np.int64)      # node -> rank
        rank_of[perm] = np.arange(LPC)
        g_of_rank = np.repeat(np.asarray(gs, dtype=np.int64), 128)
        off_of_rank = offs[np.arange(LPAD) // 128] + (np.arange(LPAD) % 128) * g_of_rank
        node_off = off_of_rank[rank_of]

        run_start = np.cumsum(deg) - deg
        within = np.arange(len(loc_s)) - run_start[loc_s]
        slot = node_off[loc_s] + within

        scol = np.full(tot_slots, -1, dtype=np.int64)
        srow = np.full(tot_slots, -1, dtype=np.int64)
        scol[slot] = col_s
        srow[slot] = loc_s + c * LPC

        deg_rank = np.zeros(LPAD, dtype=np.int64)
        deg_rank[:LPC] = deg[perm]
        pad_rank = g_of_rank - deg_rank
        invd_rank = 1.0 / np.maximum(deg_rank, 1)

        cores.append(dict(scol=scol, srow=srow, perm=perm,
                          pad_rank=pad_rank, invd_rank=invd_rank))
    return plan, cores


def _build_program(plan):
    tot = plan["tot_slots"]
    segs = plan["segments"]
    chunks = plan["chunks"]
    cum_chunks = plan["cum_chunks"]
    cum_chunks_of_seg = plan["cum_chunks_of_seg"]
    gs = plan["gs"]
    maxblock = plan["maxblock"]
    dr_eng, dr_ord = plan["dr_eng"], plan["dr_ord"]
    cum_dr = plan["cum_dr"]
    f1_eng, f1_ord = plan["f1_eng"], plan["f1_ord"]
    nseg = len(segs)
    nchunk = len(chunks)

    nc = bacc.Bacc("TRN2")
    xg_d = nc.dram_tensor("xg", [128, tot], BF16, kind="ExternalInput")
    wcat_d = nc.dram_tensor("wcat", [128, 128], BF16, kind="ExternalInput")
    bias_d = nc.dram_tensor("bias", [128, 1], F32, kind="ExternalInput")
    rout_d = nc.dram_tensor("rout", [128, LPAD], BF16, kind="ExternalOutput")

    from contextlib import ExitStack

    with ExitStack() as ctx:
        block = ctx.enter_context(nc.Block())
        sb = lambda name, shape, dt: ctx.enter_context(nc.sbuf_tensor(name, shape, dt))
        ps = lambda name, shape: ctx.enter_context(nc.psum_tensor(name, shape, F32))
        sem = lambda name: ctx.enter_context(nc.semaphore(name))

        xgs = [sb(f"xgs{i}", [128, SEG_SLOTS], BF16) for i in range(NXGBUF)]
        msb = [sb(f"msb{i}", [128, maxblock], BF16) for i in range(NMSB)]
        mf1 = [sb(f"mf1_{i}", [128, maxblock // 2], BF16) for i in range(NMF1)]
        mf2 = [sb(f"mf2_{i}", [128, maxblock // 4], BF16) for i in range(2)]
        rout_sb = sb("rout_sb", [128, LPAD], BF16)
        wcat_s = sb("wcat_s", [128, 128], BF16)
        bias_s = sb("bias_s", [128, 1], F32)
        pz = [ps("pz0", [128, CHUNK]), ps("pz1", [128, CHUNK])]

        s_in = sem("s_in")
        s_seg = [sem(f"s_seg{i}") for i in range(NXGBUF)]
        s_mm = sem("s_mm")
        s_dr = {ACT: sem("s_dr_a"), DVE: sem("s_dr_v")}
        s_f1 = {DVE: sem("s_f1_v"), POOL: sem("s_f1_p")}
        s_red = sem("s_red")
        s_done = sem("s_done")

        # first chunk (within engine e's drain sequence) of each batch
        first_dr_of_batch = {ACT: {}, DVE: {}}
        for t, (bj, si, c0, moff, w) in enumerate(chunks):
            e = dr_eng[t]
            if bj not in first_dr_of_batch[e]:
                first_dr_of_batch[e][bj] = t

        @block.sync
        def _(sync):
            sync.dma_start(wcat_s[:, :], wcat_d[:, :]).then_inc(s_in, 16)
            sync.dma_start(bias_s[:, :], bias_d[:, :]).then_inc(s_in, 16)
            # segment prefetch runs unencumbered: only buffer-recycle waits,
            # so the PE never starves at segment boundaries (HAM stays warm)
            for si, (st, ns, b0, nb) in enumerate(segs):
                if si >= NXGBUF:
                    sync.wait_ge(s_mm, cum_chunks_of_seg[si - NXGBUF])
                sync.dma_start(xgs[si % NXGBUF][:, :ns],
                               xg_d[:, st:st + ns]).then_inc(s_seg[si % NXGBUF], 16)
            # reduce outputs accumulate in rout_sb (never recycled); ship
            # them out in a few overlapped slices as reduces retire
            for j0 in range(0, NBATCH, ROUT_GRP):
                j1 = min(j0 + ROUT_GRP, NBATCH)
                sync.wait_ge(s_red, j1)
                sync.dma_start(rout_d[:, 128 * j0:128 * j1],
                               rout_sb[:, 128 * j0:128 * j1]).then_inc(s_done, 16)

        @block.tensor
        def _(pe):
            pe.wait_ge(s_in, 32)
            for t, (bj, si, c0, moff, w) in enumerate(chunks):
                pe.wait_ge(s_seg[si % NXGBUF], 16 * (si // NXGBUF + 1))
                if t >= 2:
                    tp = t - 2  # pz[t%2] free once chunk t-2 was drained
                    pe.wait_ge(s_dr[dr_eng[tp]], dr_ord[tp] + 1)
                for q0 in range(0, w, 512):
                    qw = min(512, w - q0)
                    mm = pe.matmul(pz[t % 2][:, q0:q0 + qw], wcat_s[:, :],
                                   xgs[si % NXGBUF][:, c0 + q0:c0 + q0 + qw],
                                   start=True, stop=True)
                    if q0 + qw == w:
                        mm.then_inc(s_mm)

        def emit_drain(eng, t):
            bj, si, c0, moff, w = chunks[t]
            eng.wait_ge(s_mm, t + 1)
            if first_dr_of_batch[dr_eng[t]].get(bj) == t and bj >= NMSB:
                jp = bj - NMSB  # msb[bj%NMSB] free once fold1 jp retired
                eng.wait_ge(s_f1[f1_eng[jp]], f1_ord[jp] + 1)
            if dr_eng[t] == ACT:
                eng.activation(msb[bj % NMSB][:, moff:moff + w],
                               pz[t % 2][:, :w],
                               mybir.ActivationFunctionType.Relu,
                               bias=bias_s[:, 0:1]).then_inc(s_dr[ACT])
            else:
                eng.tensor_scalar(msb[bj % NMSB][:, moff:moff + w],
                                  pz[t % 2][:, :w],
                                  bias_s[:, 0:1], 0.0,
                                  op0=mybir.AluOpType.add,
                                  op1=mybir.AluOpType.max).then_inc(s_dr[DVE])

        def emit_fold1(eng, j, self_eng):
            # mf1[j%2] free once reduce j-2 retired; drains of batch j done
            g = gs[j]
            h = g // 2
            if j >= NMF1:
                eng.wait_ge(s_red, j - NMF1 + 1)
            for e in (ACT, DVE):
                if e != self_eng and cum_dr[e][j]:
                    eng.wait_ge(s_dr[e], cum_dr[e][j])
            m = msb[j % NMSB]
            eng.tensor_tensor(
                mf1[j % NMF1][:, :128 * h].rearrange("p (n h) -> p n h", h=h),
                m[:, :128 * g].rearrange("p (n g) -> p n g", g=g)[:, :, :h],
                m[:, :128 * g].rearrange("p (n g) -> p n g", g=g)[:, :, h:],
                op=mybir.AluOpType.add,
            ).then_inc(s_f1[self_eng])

        def emit_fold2(dve, j):
            g2 = gs[j] // 2
            h = g2 // 2
            if f1_eng[j] == POOL:
                dve.wait_ge(s_f1[POOL], f1_ord[j] + 1)
            src = mf1[j % NMF1][:, :128 * g2].rearrange("p (n g) -> p n g", g=g2)
            dve.tensor_tensor(
                mf2[j % 2][:, :128 * h].rearrange("p (n h) -> p n h", h=h),
                src[:, :, :h], src[:, :, h:],
                op=mybir.AluOpType.add,
            )

        def emit_reduce(dve, j):
            g = gs[j]
            two_fold = (g // 2) % 2 == 0 and g >= 4
            gr = g // 4 if two_fold else g // 2
            src = (mf2[j % 2] if two_fold else mf1[j % NMF1])
            if not two_fold and f1_eng[j] == POOL:
                dve.wait_ge(s_f1[POOL], f1_ord[j] + 1)
            dve.tensor_reduce(
                rout_sb[:, 128 * j:128 * (j + 1)],
                src[:, :128 * gr].rearrange("p (n g) -> p n g", g=gr),
                axis=mybir.AxisListType.X,
                op=mybir.AluOpType.add,
            ).then_inc(s_red)

        def emit_steps(dve, j):
            if f1_eng[j] == DVE:
                emit_fold1(dve, j, DVE)
            if (gs[j] // 2) % 2 == 0 and gs[j] >= 4:
                emit_fold2(dve, j)
            emit_reduce(dve, j)

        @block.scalar
        def _(act):
            act.wait_ge(s_in, 32)
            for t in range(nchunk):
                if dr_eng[t] == ACT:
                    emit_drain(act, t)

        @block.vector
        def _(dve):
            dve.wait_ge(s_in, 32)
            with nc.allow_low_precision(reason="bf16 rounding of final sums"):
                done = 0
                for t in range(nchunk):
                    bj = chunks[t][0]
                    while done < bj:
                        emit_steps(dve, done)
                        done += 1
                    if dr_eng[t] == DVE:
                        emit_drain(dve, t)
                while done < NBATCH:
                    emit_steps(dve, done)
                    done += 1

        @block.gpsimd
        def _(gp):
            gp.load_library(standard_lib)
            with nc.allow_low_precision(reason="bf16 folds"):
                for j in range(NBATCH):
                    if f1_eng[j] == POOL:
                        emit_fold1(gp, j, POOL)

    nc.compile()
    return nc


_CACHE = {}
TRACE = False
LAST_EXEC_NS = None
LAST_PROFILE_JSON = None
LAST_TRACE_PATH = None


def kernel(x, edge_index, W, b):
    x = np.asarray(x, dtype=np.float32)
    W = np.asarray(W, dtype=np.float32)
    b = np.asarray(b, dtype=np.float32)
    plan, cores = _plan_and_pack(edge_index)

    key = tuple(plan["gs"])
    if key not in _CACHE:
        _CACHE[key] = _build_program(plan)
    nc = _CACHE[key]

    # stationary weights: rows 0-63 = B = W2.T (x_col), 64-127 = A (x_row)
    W1, W2 = W[:, :F_IN], W[:, F_IN:]
    wcat = np.zeros((128, F_OUT), dtype=np.float32)
    wcat[:F_IN] = W2.T
    wcat[F_IN:] = (W1 - W2).T
    wcat = wcat.astype(ml_dtypes.bfloat16)
    bias = b.reshape(128, 1).astype(np.float32)
    relu_b = np.maximum(b, 0.0)

    xbT = np.ascontiguousarray(x.astype(ml_dtypes.bfloat16).T)  # [64, N]

    tot = plan["tot_slots"]
    in_maps = []
    for c in range(NCORES):
        pc = cores[c]
        scol, srow = pc["scol"], pc["srow"]
        xg = np.zeros((128, tot), dtype=ml_dtypes.bfloat16)
        vs = np.flatnonzero(scol >= 0)
        xg[:F_IN, vs] = xbT[:, scol[vs]]
        xg[F_IN:, vs] = xbT[:, srow[vs]]
        in_maps.append({"xg": xg, "wcat": wcat, "bias": bias})

    global LAST_EXEC_NS, LAST_PROFILE_JSON, LAST_TRACE_PATH
    res = run_bass_kernel_spmd(nc, in_maps, core_ids=list(range(NCORES)),
                               trace=TRACE)
    if TRACE:
        LAST_EXEC_NS = res.exec_time_ns
        LAST_PROFILE_JSON = res.profile_json
        if res.instructions_and_trace is not None:
            LAST_TRACE_PATH = res.instructions_and_trace[1]

    # ---- assembly
    out = np.zeros((N_NODES, F_OUT), dtype=np.float32)
    for c in range(NCORES):
        pc = cores[c]
        R = res.results[c]["rout"].astype(np.float32).T   # [LPAD, 128]
        R = R[:LPC] - pc["pad_rank"][:LPC, None] * relu_b[None, :]
        R *= pc["invd_rank"][:LPC, None]
        out[pc["perm"] + c * LPC] = R
    return out
